# revision 9
# baseline (speedup 1.0000x reference)
"""AdjacencyMatchingLoss on 8 trn2 NeuronCores — self-contained.

Math (per batch b):
    A[p,q] = (d_hw[p,q] == 1)
    loss   = -mean_b( sum_e w_be * (P_b A)[src_be] . P_b[dst_be] / max(sum_e w_be, eps) )

Gather-free rewrite (all matmuls, no transposes):
    wt_be     = w_be / max(sum_e w_be, eps)
    Wt_b[j,i] = sum_e wt_be * [dst_be==j][src_be==i]     (one-hot matmuls over e)
    U_b[i,q]  = sum_j Wt_b[j,i] P_b[j,q]                 (matmul)
    V[p,q]    = sum_b sum_i P_b[i,p] U_b[i,q]            (matmul, K stacked over b)
    partial   = sum_{p,q} A[p,q] V[p,q]                  (DVE mult + ACT accum)
    loss      = -(1/B) sum_cores partial

Data-parallel over batch: 2 batches/core, d_hw replicated, host sums 8 scalars.
"""

import numpy as np

B, NLOG, NPHYS, E = 16, 512, 2048, 2048
NCORES = 8
BLOC = B // NCORES          # batches per core
NI = NLOG // 128            # 4  i/j-chunks per batch
NE = E // 128               # 16 e-chunks per batch
NP = NPHYS // 128           # 16 p-chunks
NQ = NPHYS // 512           # 4  q-chunks of 512
KV = BLOC * NI              # 8  K-chunks for the V matmul
EPS = 1e-8

_CACHE = {}


def _emit(tc, aps, dt_a_name="float8e4"):
    from contextlib import ExitStack

    from concourse import mybir
    from concourse.bass_isa import ReduceOp

    nc = tc.nc
    f32 = mybir.dt.float32
    f16 = mybir.dt.float16
    i32 = mybir.dt.int32
    dt_a = getattr(mybir.dt, dt_a_name)
    AO = mybir.AluOpType
    ACT_COPY = mybir.ActivationFunctionType.Copy

    P_ap = aps["P"]
    d_ap = aps["d_hw"]
    src_ap = aps["edge_src"]
    dst_ap = aps["edge_dst"]
    w_ap = aps["edge_w"]
    out_ap = aps["out"]

    ctx = ExitStack()
    with ctx:
        const = ctx.enter_context(tc.tile_pool(name="const", bufs=1))
        pstage = ctx.enter_context(tc.tile_pool(name="pstage", bufs=2))
        dstage = ctx.enter_context(tc.tile_pool(name="dstage", bufs=4))
        big = ctx.enter_context(tc.tile_pool(name="big", bufs=1))
        wtp = ctx.enter_context(tc.tile_pool(name="wtp", bufs=2))
        ohp = ctx.enter_context(tc.tile_pool(name="ohp", bufs=3))
        astr = ctx.enter_context(tc.tile_pool(name="astr", bufs=6))
        edg = ctx.enter_context(tc.tile_pool(name="edg", bufs=2))
        accp = ctx.enter_context(tc.tile_pool(name="accp", bufs=3))
        scr = ctx.enter_context(tc.tile_pool(name="scr", bufs=3))
        psum = ctx.enter_context(tc.tile_pool(name="psum", bufs=2, space="PSUM"))

        # constants
        iota_i = const.tile([128, 512], i32)
        nc.gpsimd.iota(iota_i, pattern=[[1, 512]], base=0, channel_multiplier=0)
        iota_f = const.tile([128, 512], f32)
        nc.vector.tensor_copy(iota_f, iota_i)

        # ---- load P, convert to fp16, stacked [128, KV, NPHYS] ----
        P16 = big.tile([128, KV, NPHYS], f16, tag="P16")
        for b in range(BLOC):
            for ic in range(NI):
                st = pstage.tile([128, NPHYS], f32, tag="pstage")
                nc.sync.dma_start(st, P_ap[b, ic * 128:(ic + 1) * 128, :])
                nc.vector.tensor_copy(P16[:, b * NI + ic, :], st)

        U16 = big.tile([128, KV, NPHYS], f16, tag="U16")

        # ---- per-batch: edges -> one-hots -> Wt -> U ----
        for b in range(BLOC):
            src32 = edg.tile([128, NE], i32, tag="src32")
            nc.sync.dma_start(src32, src_ap[b].rearrange("(p c) -> p c", p=128))
            dst32 = edg.tile([128, NE], i32, tag="dst32")
            nc.sync.dma_start(dst32, dst_ap[b].rearrange("(p c) -> p c", p=128))
            wf = edg.tile([128, NE], f32, tag="wf")
            nc.sync.dma_start(wf, w_ap[b].rearrange("(p c) -> p c", p=128))

            srcf = edg.tile([128, NE], f32, tag="srcf")
            nc.vector.tensor_copy(srcf, src32)
            dstf = edg.tile([128, NE], f32, tag="dstf")
            nc.vector.tensor_copy(dstf, dst32)

            # sample_weight = max(sum(w), eps) broadcast over partitions
            swp = edg.tile([128, 1], f32, tag="swp")
            nc.vector.tensor_reduce(swp, wf, axis=mybir.AxisListType.X, op=AO.add)
            swa = edg.tile([128, 1], f32, tag="swa")
            nc.gpsimd.partition_all_reduce(swa, swp, 128, ReduceOp.add)
            swm = edg.tile([128, 1], f32, tag="swm")
            nc.vector.tensor_scalar_max(swm, swa, EPS)
            rsw = edg.tile([128, 1], f32, tag="rsw")
            nc.vector.reciprocal(rsw, swm)
            wtf = edg.tile([128, NE], f32, tag="wtf")
            nc.vector.tensor_scalar_mul(wtf, wf, rsw)

            # Wt[j,i]: 4 j-banks side by side in one psum tile
            ps_wt = psum.tile([128, 2048], f32, tag="ps")
            for c in range(NE):
                od = ohp.tile([128, 512], f16, tag="od")
                nc.vector.tensor_scalar(
                    od, iota_f, dstf[:, c:c + 1], None, op0=AO.is_equal
                )
                osw = ohp.tile([128, 512], f16, tag="osw")
                nc.vector.tensor_scalar(
                    osw, iota_f, srcf[:, c:c + 1], wtf[:, c:c + 1],
                    op0=AO.is_equal, op1=AO.mult,
                )
                for j in range(NI):
                    nc.tensor.matmul(
                        ps_wt[:, j * 512:(j + 1) * 512],
                        od[:, j * 128:(j + 1) * 128],
                        osw,
                        start=(c == 0),
                        stop=(c == NE - 1),
                    )
            wt16 = wtp.tile([128, NI, 512], f16, tag="wt16")
            nc.scalar.copy(wt16, ps_wt.rearrange("p (j i) -> p j i", j=NI))

            # U_b[i,q] = sum_j Wt[j,i] P_b[j,q]
            for ic in range(NI):
                ps_u = psum.tile([128, 2048], f32, tag="ps")
                for j in range(NI):
                    for qc in range(NQ):
                        nc.tensor.matmul(
                            ps_u[:, qc * 512:(qc + 1) * 512],
                            wt16[:, j, ic * 128:(ic + 1) * 128],
                            P16[:, b * NI + j, qc * 512:(qc + 1) * 512],
                            start=(j == 0),
                            stop=(j == NI - 1),
                        )
                nc.scalar.copy(U16[:, b * NI + ic, :], ps_u)

        # ---- V = P^T U (K = KV*128), consumed tile-by-tile against A ----
        acc = None
        for pc in range(NP):
            ps_v = psum.tile([128, 2048], f32, tag="ps")
            for kc in range(KV):
                for qc in range(NQ):
                    nc.tensor.matmul(
                        ps_v[:, qc * 512:(qc + 1) * 512],
                        P16[:, kc, pc * 128:(pc + 1) * 128],
                        U16[:, kc, qc * 512:(qc + 1) * 512],
                        start=(kc == 0),
                        stop=(kc == KV - 1),
                    )
            # A chunk stream: (d_hw[p-chunk] == 1)
            dint = dstage.tile([128, NPHYS], i32, tag="dstage")
            nc.sync.dma_start(dint, d_ap[pc * 128:(pc + 1) * 128, :])
            ach = astr.tile([128, NPHYS], dt_a, tag="ach")
            nc.gpsimd.tensor_scalar(ach, dint, 1, None, op0=AO.is_equal)
            for qc in range(NQ):
                tmp = scr.tile([128, 512], f16, tag="tmp")
                nc.vector.tensor_tensor(
                    tmp, ps_v[:, qc * 512:(qc + 1) * 512],
                    ach[:, qc * 512:(qc + 1) * 512], AO.mult,
                )
                pacc = accp.tile([128, 1], f32, tag="pacc")
                trash = scr.tile([128, 512], f16, tag="trash")
                nc.scalar.activation(trash, tmp, ACT_COPY, accum_out=pacc)
                if acc is None:
                    acc = pacc
                else:
                    nacc = accp.tile([128, 1], f32, tag="acc")
                    nc.scalar.add(nacc, pacc, acc[:, 0:1])
                    acc = nacc

        # ---- partition-reduce to a scalar, write out ----
        fin = const.tile([128, 1], f32)
        nc.gpsimd.partition_all_reduce(fin, acc, 128, ReduceOp.add)
        res = const.tile([1, 1], f32)
        nc.vector.tensor_copy(res, fin[0:1, 0:1])
        nc.sync.dma_start(out_ap, res)


def build(dt_a_name="float8e4"):
    import concourse.tile as tile
    from concourse import bacc, mybir

    f32 = mybir.dt.float32
    i32 = mybir.dt.int32
    nc = bacc.Bacc(
        "TRN2", target_bir_lowering=False, debug=False, num_devices=NCORES
    )
    aps = {
        "P": nc.dram_tensor("P", [BLOC, NLOG, NPHYS], f32, kind="ExternalInput").ap(),
        "d_hw": nc.dram_tensor("d_hw", [NPHYS, NPHYS], i32, kind="ExternalInput").ap(),
        "edge_src": nc.dram_tensor("edge_src", [BLOC, E], i32, kind="ExternalInput").ap(),
        "edge_dst": nc.dram_tensor("edge_dst", [BLOC, E], i32, kind="ExternalInput").ap(),
        "edge_w": nc.dram_tensor("edge_w", [BLOC, E], f32, kind="ExternalInput").ap(),
        "out": nc.dram_tensor("out", [1, 1], f32, kind="ExternalOutput").ap(),
    }
    with tile.TileContext(nc) as tc:
        _emit(tc, aps, dt_a_name)
    nc.compile()
    return nc


def shard_inputs(P, d_hw, edge_src, edge_dst, edge_w):
    P = np.ascontiguousarray(np.asarray(P, dtype=np.float32))
    d_hw = np.ascontiguousarray(np.asarray(d_hw, dtype=np.int32))
    edge_src = np.ascontiguousarray(np.asarray(edge_src, dtype=np.int32))
    edge_dst = np.ascontiguousarray(np.asarray(edge_dst, dtype=np.int32))
    edge_w = np.ascontiguousarray(np.asarray(edge_w, dtype=np.float32))
    in_maps = []
    for c in range(NCORES):
        sl = slice(c * BLOC, (c + 1) * BLOC)
        in_maps.append(
            {
                "P": P[sl],
                "d_hw": d_hw,
                "edge_src": edge_src[sl],
                "edge_dst": edge_dst[sl],
                "edge_w": edge_w[sl],
            }
        )
    return in_maps


def kernel(P, d_hw, edge_src, edge_dst, edge_w):
    from concourse.bass_utils import run_bass_kernel_spmd

    if "nc" not in _CACHE:
        _CACHE["nc"] = build()
    nc = _CACHE["nc"]
    in_maps = shard_inputs(P, d_hw, edge_src, edge_dst, edge_w)
    res = run_bass_kernel_spmd(nc, in_maps, core_ids=list(range(NCORES)))
    partial = sum(float(res.results[c]["out"][0, 0]) for c in range(NCORES))
    return np.float32(-partial / B)


# revision 10
# speedup vs baseline: 103.6486x; 103.6486x over previous
"""AdjacencyMatchingLoss on 8 trn2 NeuronCores — self-contained.

Math (per batch b):
    A[p,q] = (d_hw[p,q] == 1)
    loss   = -mean_b( sum_e w_be * (P_b A)[src_be] . P_b[dst_be] / max(sum_e w_be, eps) )

Gather-free rewrite (all matmuls, no transposes):
    wt_be     = w_be / max(sum_e w_be, eps)
    Wt_b[j,i] = sum_e wt_be * [dst_be==j][src_be==i]     (one-hot matmuls over e)
    U_b[i,q]  = sum_j Wt_b[j,i] P_b[j,q]                 (matmul)
    V[p,q]    = sum_b sum_i P_b[i,p] U_b[i,q]            (matmul, K stacked over b)
    partial   = sum_{p,q} A[p,q] V[p,q]                  (DVE mult + ACT accum)
    loss      = -(1/B) sum_cores partial

Data-parallel over batch: 2 batches/core, d_hw replicated, host sums 8 scalars.
"""

import numpy as np

B, NLOG, NPHYS, E = 16, 512, 2048, 2048
NCORES = 8
BLOC = B // NCORES          # batches per core
NI = NLOG // 128            # 4  i/j-chunks per batch
NE = E // 128               # 16 e-chunks per batch
NP = NPHYS // 128           # 16 p-chunks
NQ = NPHYS // 512           # 4  q-chunks of 512
KV = BLOC * NI              # 8  K-chunks for the V matmul
EPS = 1e-8

_CACHE = {}


def _emit(tc, aps, dt_a_name="float8e4"):
    from contextlib import ExitStack

    from concourse import mybir
    from concourse.bass_isa import ReduceOp

    nc = tc.nc
    f32 = mybir.dt.float32
    f16 = mybir.dt.float16
    i32 = mybir.dt.int32
    dt_a = getattr(mybir.dt, dt_a_name)
    AO = mybir.AluOpType
    ACT_COPY = mybir.ActivationFunctionType.Copy

    P_ap = aps["P"]
    d_ap = aps["d_hw"]
    src_ap = aps["edge_src"]
    dst_ap = aps["edge_dst"]
    w_ap = aps["edge_w"]
    out_ap = aps["out"]

    ctx = ExitStack()
    with ctx:
        const = ctx.enter_context(tc.tile_pool(name="const", bufs=1))
        pstage = ctx.enter_context(tc.tile_pool(name="pstage", bufs=2))
        dstage = ctx.enter_context(tc.tile_pool(name="dstage", bufs=4))
        big = ctx.enter_context(tc.tile_pool(name="big", bufs=1))
        wtp = ctx.enter_context(tc.tile_pool(name="wtp", bufs=2))
        ohp = ctx.enter_context(tc.tile_pool(name="ohp", bufs=3))
        astr = ctx.enter_context(tc.tile_pool(name="astr", bufs=6))
        edg = ctx.enter_context(tc.tile_pool(name="edg", bufs=2))
        accp = ctx.enter_context(tc.tile_pool(name="accp", bufs=3))
        scr = ctx.enter_context(tc.tile_pool(name="scr", bufs=3))
        psum = ctx.enter_context(tc.tile_pool(name="psum", bufs=2, space="PSUM"))

        # constants
        iota_i = const.tile([128, 512], i32)
        nc.gpsimd.iota(iota_i, pattern=[[1, 512]], base=0, channel_multiplier=0)
        iota_f = const.tile([128, 512], f32)
        nc.vector.tensor_copy(iota_f, iota_i)

        # ---- load P, convert to fp16, stacked [128, KV, NPHYS] ----
        P16 = big.tile([128, KV, NPHYS], f16, tag="P16")
        for b in range(BLOC):
            for ic in range(NI):
                st = pstage.tile([128, NPHYS], f32, tag="pstage")
                nc.sync.dma_start(st, P_ap[b, ic * 128:(ic + 1) * 128, :])
                nc.vector.tensor_copy(P16[:, b * NI + ic, :], st)

        U16 = big.tile([128, KV, NPHYS], f16, tag="U16")

        # ---- per-batch: edges -> one-hots -> Wt -> U ----
        for b in range(BLOC):
            src32 = edg.tile([128, NE], i32, tag="src32")
            nc.sync.dma_start(src32, src_ap[b].rearrange("(p c) -> p c", p=128))
            dst32 = edg.tile([128, NE], i32, tag="dst32")
            nc.sync.dma_start(dst32, dst_ap[b].rearrange("(p c) -> p c", p=128))
            wf = edg.tile([128, NE], f32, tag="wf")
            nc.sync.dma_start(wf, w_ap[b].rearrange("(p c) -> p c", p=128))

            srcf = edg.tile([128, NE], f32, tag="srcf")
            nc.vector.tensor_copy(srcf, src32)
            dstf = edg.tile([128, NE], f32, tag="dstf")
            nc.vector.tensor_copy(dstf, dst32)

            # sample_weight = max(sum(w), eps) broadcast over partitions
            swp = edg.tile([128, 1], f32, tag="swp")
            nc.vector.tensor_reduce(swp, wf, axis=mybir.AxisListType.X, op=AO.add)
            swa = edg.tile([128, 1], f32, tag="swa")
            nc.gpsimd.partition_all_reduce(swa, swp, 128, ReduceOp.add)
            swm = edg.tile([128, 1], f32, tag="swm")
            nc.vector.tensor_scalar_max(swm, swa, EPS)
            rsw = edg.tile([128, 1], f32, tag="rsw")
            nc.vector.reciprocal(rsw, swm)
            wtf = edg.tile([128, NE], f32, tag="wtf")
            nc.vector.tensor_scalar_mul(wtf, wf, rsw)

            # Wt[j,i]: 4 j-banks side by side in one psum tile
            ps_wt = psum.tile([128, 2048], f32, tag="ps")
            for c in range(NE):
                od = ohp.tile([128, 512], f16, tag="od")
                nc.vector.tensor_scalar(
                    od, iota_f, dstf[:, c:c + 1], None, op0=AO.is_equal
                )
                osw = ohp.tile([128, 512], f16, tag="osw")
                nc.vector.tensor_scalar(
                    osw, iota_f, srcf[:, c:c + 1], wtf[:, c:c + 1],
                    op0=AO.is_equal, op1=AO.mult,
                )
                for j in range(NI):
                    nc.tensor.matmul(
                        ps_wt[:, j * 512:(j + 1) * 512],
                        od[:, j * 128:(j + 1) * 128],
                        osw,
                        start=(c == 0),
                        stop=(c == NE - 1),
                    )
            wt16 = wtp.tile([128, NI, 512], f16, tag="wt16")
            nc.scalar.copy(wt16, ps_wt.rearrange("p (j i) -> p j i", j=NI))

            # U_b[i,q] = sum_j Wt[j,i] P_b[j,q]
            for ic in range(NI):
                ps_u = psum.tile([128, 2048], f32, tag="ps")
                for j in range(NI):
                    for qc in range(NQ):
                        nc.tensor.matmul(
                            ps_u[:, qc * 512:(qc + 1) * 512],
                            wt16[:, j, ic * 128:(ic + 1) * 128],
                            P16[:, b * NI + j, qc * 512:(qc + 1) * 512],
                            start=(j == 0),
                            stop=(j == NI - 1),
                        )
                nc.scalar.copy(U16[:, b * NI + ic, :], ps_u)

        # ---- V = P^T U (K = KV*128), consumed tile-by-tile against A ----
        acc = None
        for pc in range(NP):
            ps_v = psum.tile([128, 2048], f32, tag="ps")
            for kc in range(KV):
                for qc in range(NQ):
                    nc.tensor.matmul(
                        ps_v[:, qc * 512:(qc + 1) * 512],
                        P16[:, kc, pc * 128:(pc + 1) * 128],
                        U16[:, kc, qc * 512:(qc + 1) * 512],
                        start=(kc == 0),
                        stop=(kc == KV - 1),
                    )
            # A chunk stream: (d_hw[p-chunk] == 1)
            dint = dstage.tile([128, NPHYS], i32, tag="dstage")
            nc.sync.dma_start(dint, d_ap[pc * 128:(pc + 1) * 128, :])
            ach = astr.tile([128, NPHYS], dt_a, tag="ach")
            nc.gpsimd.tensor_scalar(ach, dint, 1, None, op0=AO.is_equal)
            for qc in range(NQ):
                tmp = scr.tile([128, 512], f16, tag="tmp")
                nc.vector.tensor_tensor(
                    tmp, ps_v[:, qc * 512:(qc + 1) * 512],
                    ach[:, qc * 512:(qc + 1) * 512], AO.mult,
                )
                pacc = accp.tile([128, 1], f32, tag="pacc")
                trash = scr.tile([128, 512], f16, tag="trash")
                nc.scalar.activation(trash, tmp, ACT_COPY, accum_out=pacc)
                if acc is None:
                    acc = pacc
                else:
                    nacc = accp.tile([128, 1], f32, tag="acc")
                    nc.scalar.add(nacc, pacc, acc[:, 0:1])
                    acc = nacc

        # ---- partition-reduce to a scalar, write out ----
        fin = const.tile([128, 1], f32)
        nc.gpsimd.partition_all_reduce(fin, acc, 128, ReduceOp.add)
        res = const.tile([1, 1], f32)
        nc.vector.tensor_copy(res, fin[0:1, 0:1])
        nc.sync.dma_start(out_ap, res)


def build(dt_a_name="float8e4", repeat=1):
    import concourse.tile as tile
    from concourse import bacc, mybir

    f32 = mybir.dt.float32
    i32 = mybir.dt.int32
    nc = bacc.Bacc(
        "TRN2", target_bir_lowering=False, debug=False, num_devices=NCORES
    )
    aps = {
        "P": nc.dram_tensor("P", [BLOC, NLOG, NPHYS], f32, kind="ExternalInput").ap(),
        "d_hw": nc.dram_tensor("d_hw", [NPHYS, NPHYS], i32, kind="ExternalInput").ap(),
        "edge_src": nc.dram_tensor("edge_src", [BLOC, E], i32, kind="ExternalInput").ap(),
        "edge_dst": nc.dram_tensor("edge_dst", [BLOC, E], i32, kind="ExternalInput").ap(),
        "edge_w": nc.dram_tensor("edge_w", [BLOC, E], f32, kind="ExternalInput").ap(),
        "out": nc.dram_tensor("out", [1, 1], f32, kind="ExternalOutput").ap(),
    }
    with tile.TileContext(nc) as tc:
        for _ in range(repeat):
            _emit(tc, aps, dt_a_name)
    nc.compile()
    return nc


def shard_inputs(P, d_hw, edge_src, edge_dst, edge_w):
    P = np.ascontiguousarray(np.asarray(P, dtype=np.float32))
    d_hw = np.ascontiguousarray(np.asarray(d_hw, dtype=np.int32))
    edge_src = np.ascontiguousarray(np.asarray(edge_src, dtype=np.int32))
    edge_dst = np.ascontiguousarray(np.asarray(edge_dst, dtype=np.int32))
    edge_w = np.ascontiguousarray(np.asarray(edge_w, dtype=np.float32))
    in_maps = []
    for c in range(NCORES):
        sl = slice(c * BLOC, (c + 1) * BLOC)
        in_maps.append(
            {
                "P": P[sl],
                "d_hw": d_hw,
                "edge_src": edge_src[sl],
                "edge_dst": edge_dst[sl],
                "edge_w": edge_w[sl],
            }
        )
    return in_maps


def kernel(P, d_hw, edge_src, edge_dst, edge_w):
    from concourse.bass_utils import run_bass_kernel_spmd

    if "nc" not in _CACHE:
        _CACHE["nc"] = build()
    nc = _CACHE["nc"]
    in_maps = shard_inputs(P, d_hw, edge_src, edge_dst, edge_w)
    res = run_bass_kernel_spmd(nc, in_maps, core_ids=list(range(NCORES)))
    partial = sum(float(res.results[c]["out"][0, 0]) for c in range(NCORES))
    return np.float32(-partial / B)


# revision 11
# speedup vs baseline: 448.5936x; 4.3280x over previous
"""AdjacencyMatchingLoss on 8 trn2 NeuronCores — self-contained.

Math (per batch b):
    A[p,q] = (d_hw[p,q] == 1)
    loss   = -mean_b( sum_e w_be * (P_b A)[src_be] . P_b[dst_be] / max(sum_e w_be, eps) )

Gather-free rewrite (all matmuls, no transposes):
    wt_be     = w_be / max(sum_e w_be, eps)
    Wt_b[j,i] = sum_e wt_be * [dst_be==j][src_be==i]     (one-hot matmuls over e)
    U_b[i,q]  = sum_j Wt_b[j,i] P_b[j,q]                 (matmul)
    V[p,q]    = sum_b sum_i P_b[i,p] U_b[i,q]            (matmul, K stacked over b)
    partial   = sum_{p,q} A[p,q] V[p,q]                  (DVE mult + ACT accum)
    loss      = -(1/B) sum_cores partial

Data-parallel over batch: 2 batches/core, d_hw replicated, host sums 8 scalars.
"""

import numpy as np

B, NLOG, NPHYS, E = 16, 512, 2048, 2048
NCORES = 8
BLOC = B // NCORES          # batches per core
NI = NLOG // 128            # 4  i/j-chunks per batch
NE = E // 128               # 16 e-chunks per batch
NP = NPHYS // 128           # 16 p-chunks
NQ = NPHYS // 512           # 4  q-chunks of 512
KV = BLOC * NI              # 8  K-chunks for the V matmul
EPS = 1e-8

_CACHE = {}


def _emit(tc, aps, dt_a_name="float8e4"):
    from contextlib import ExitStack

    from concourse import mybir

    nc = tc.nc
    f32 = mybir.dt.float32
    f16 = mybir.dt.float16
    i32 = mybir.dt.int32
    dt_a = getattr(mybir.dt, dt_a_name)
    AO = mybir.AluOpType
    ACT_COPY = mybir.ActivationFunctionType.Copy

    P_ap = aps["P"]
    d_ap = aps["d_hw"]
    src_ap = aps["edge_src"]
    dst_ap = aps["edge_dst"]
    w_ap = aps["edge_w"]
    iota_ap = aps["iota"]      # [128, 512] f16: row 0..511 on every partition
    ones_ap = aps["ones"]      # [128, 128] f32: all ones
    out_ap = aps["out"]

    ctx = ExitStack()
    with ctx:
        const = ctx.enter_context(tc.tile_pool(name="const", bufs=1))
        pstage = ctx.enter_context(tc.tile_pool(name="pstage", bufs=2))
        dstage = ctx.enter_context(tc.tile_pool(name="dstage", bufs=4))
        big = ctx.enter_context(tc.tile_pool(name="big", bufs=1))
        wtp = ctx.enter_context(tc.tile_pool(name="wtp", bufs=2))
        ohp = ctx.enter_context(tc.tile_pool(name="ohp", bufs=4))
        astr = ctx.enter_context(tc.tile_pool(name="astr", bufs=4))
        edg = ctx.enter_context(tc.tile_pool(name="edg", bufs=2))
        accp = ctx.enter_context(tc.tile_pool(name="accp", bufs=3))
        scr = ctx.enter_context(tc.tile_pool(name="scr", bufs=3))
        psum = ctx.enter_context(tc.tile_pool(name="psum", bufs=2, space="PSUM"))

        # host-provided constants
        iota16 = const.tile([128, 512], f16)
        nc.sync.dma_start(iota16, iota_ap)
        ones = const.tile([128, 128], f32)
        nc.sync.dma_start(ones, ones_ap)

        # ---- load P, convert to fp16 on ACT, stacked [128, KV, NPHYS] ----
        P16 = big.tile([128, KV, NPHYS], f16, tag="P16")
        for b in range(BLOC):
            for ic in range(NI):
                st = pstage.tile([128, NPHYS], f32, tag="pstage")
                nc.sync.dma_start(st, P_ap[b, ic * 128:(ic + 1) * 128, :])
                nc.scalar.copy(P16[:, b * NI + ic, :], st)

        U16 = big.tile([128, KV, NPHYS], f16, tag="U16")

        # ---- per-batch: edges -> one-hots -> Wt -> U ----
        for b in range(BLOC):
            src32 = edg.tile([128, NE], i32, tag="src32")
            nc.sync.dma_start(src32, src_ap[b].rearrange("(p c) -> p c", p=128))
            dst32 = edg.tile([128, NE], i32, tag="dst32")
            nc.sync.dma_start(dst32, dst_ap[b].rearrange("(p c) -> p c", p=128))
            wf = edg.tile([128, NE], f32, tag="wf")
            nc.sync.dma_start(wf, w_ap[b].rearrange("(p c) -> p c", p=128))

            srcf = edg.tile([128, NE], f32, tag="srcf")
            nc.vector.tensor_copy(srcf, src32)
            dstf = edg.tile([128, NE], f32, tag="dstf")
            nc.vector.tensor_copy(dstf, dst32)

            # sample_weight = max(sum(w), eps), broadcast via ones-matmul
            swp = edg.tile([128, 1], f32, tag="swp")
            nc.vector.tensor_reduce(swp, wf, axis=mybir.AxisListType.X, op=AO.add)
            ps_sw = psum.tile([128, 2048], f32, tag="ps")
            nc.tensor.matmul(ps_sw[:, 0:1], ones, swp)
            swm = edg.tile([128, 1], f32, tag="swm")
            nc.vector.tensor_scalar_max(swm, ps_sw[:, 0:1], EPS)
            rsw = edg.tile([128, 1], f32, tag="rsw")
            nc.vector.reciprocal(rsw, swm)
            wtf = edg.tile([128, NE], f32, tag="wtf")
            nc.vector.tensor_scalar_mul(wtf, wf, rsw)

            # Wt[j,i]: 4 j-banks side by side in one psum tile
            ps_wt = psum.tile([128, 2048], f32, tag="ps")
            for c in range(NE):
                od = ohp.tile([128, 512], f16, tag="od")
                nc.vector.tensor_scalar(
                    od, iota16, dstf[:, c:c + 1], None, op0=AO.is_equal
                )
                osw = ohp.tile([128, 512], f16, tag="osw")
                nc.vector.tensor_scalar(
                    osw, iota16, srcf[:, c:c + 1], wtf[:, c:c + 1],
                    op0=AO.is_equal, op1=AO.mult,
                )
                for j in range(NI):
                    nc.tensor.matmul(
                        ps_wt[:, j * 512:(j + 1) * 512],
                        od[:, j * 128:(j + 1) * 128],
                        osw,
                        start=(c == 0),
                        stop=(c == NE - 1),
                    )
            wt16 = wtp.tile([128, NI, 512], f16, tag="wt16")
            nc.scalar.copy(wt16, ps_wt.rearrange("p (j i) -> p j i", j=NI))

            # U_b[i,q] = sum_j Wt[j,i] P_b[j,q]
            for ic in range(NI):
                ps_u = psum.tile([128, 2048], f32, tag="ps")
                for j in range(NI):
                    for qc in range(NQ):
                        nc.tensor.matmul(
                            ps_u[:, qc * 512:(qc + 1) * 512],
                            wt16[:, j, ic * 128:(ic + 1) * 128],
                            P16[:, b * NI + j, qc * 512:(qc + 1) * 512],
                            start=(j == 0),
                            stop=(j == NI - 1),
                        )
                nc.scalar.copy(U16[:, b * NI + ic, :], ps_u)

        # ---- V = P^T U (K = KV*128), consumed tile-by-tile against A ----
        acc = None
        for pc in range(NP):
            ps_v = psum.tile([128, 2048], f32, tag="ps")
            for kc in range(KV):
                for qc in range(NQ):
                    nc.tensor.matmul(
                        ps_v[:, qc * 512:(qc + 1) * 512],
                        P16[:, kc, pc * 128:(pc + 1) * 128],
                        U16[:, kc, qc * 512:(qc + 1) * 512],
                        start=(kc == 0),
                        stop=(kc == KV - 1),
                    )
            # A chunk stream: (d_hw[p-chunk] == 1) on DVE
            dint = dstage.tile([128, NPHYS], i32, tag="dstage")
            nc.sync.dma_start(dint, d_ap[pc * 128:(pc + 1) * 128, :])
            ach = astr.tile([128, NPHYS], dt_a, tag="ach")
            nc.vector.tensor_scalar(ach, dint, 1, None, op0=AO.is_equal)
            tmp = scr.tile([128, NPHYS], f16, tag="tmp")
            for qc in range(NQ):
                nc.vector.tensor_tensor(
                    tmp[:, qc * 512:(qc + 1) * 512],
                    ps_v[:, qc * 512:(qc + 1) * 512],
                    ach[:, qc * 512:(qc + 1) * 512], AO.mult,
                )
            pacc = accp.tile([128, 1], f32, tag="pacc")
            trash = scr.tile([128, NPHYS], f16, tag="trash")
            nc.scalar.activation(trash, tmp, ACT_COPY, accum_out=pacc)
            if acc is None:
                acc = pacc
            else:
                nacc = accp.tile([128, 1], f32, tag="acc")
                nc.scalar.add(nacc, pacc, acc[:, 0:1])
                acc = nacc

        # ---- partition-reduce via ones-matmul broadcast, write out ----
        ps_f = psum.tile([128, 2048], f32, tag="ps")
        nc.tensor.matmul(ps_f[:, 0:1], ones, acc)
        res = const.tile([1, 1], f32)
        nc.vector.tensor_copy(res, ps_f[0:1, 0:1])
        nc.sync.dma_start(out_ap, res)


def build(dt_a_name="float8e4", repeat=1):
    import concourse.tile as tile
    from concourse import bacc, mybir

    f32 = mybir.dt.float32
    i32 = mybir.dt.int32
    nc = bacc.Bacc(
        "TRN2", target_bir_lowering=False, debug=False, num_devices=NCORES
    )
    aps = {
        "P": nc.dram_tensor("P", [BLOC, NLOG, NPHYS], f32, kind="ExternalInput").ap(),
        "d_hw": nc.dram_tensor("d_hw", [NPHYS, NPHYS], i32, kind="ExternalInput").ap(),
        "edge_src": nc.dram_tensor("edge_src", [BLOC, E], i32, kind="ExternalInput").ap(),
        "edge_dst": nc.dram_tensor("edge_dst", [BLOC, E], i32, kind="ExternalInput").ap(),
        "edge_w": nc.dram_tensor("edge_w", [BLOC, E], f32, kind="ExternalInput").ap(),
        "iota": nc.dram_tensor("iota", [128, 512], mybir.dt.float16, kind="ExternalInput").ap(),
        "ones": nc.dram_tensor("ones", [128, 128], f32, kind="ExternalInput").ap(),
        "out": nc.dram_tensor("out", [1, 1], f32, kind="ExternalOutput").ap(),
    }
    with tile.TileContext(nc) as tc:
        for _ in range(repeat):
            _emit(tc, aps, dt_a_name)
    nc.compile()
    return nc


def shard_inputs(P, d_hw, edge_src, edge_dst, edge_w):
    P = np.ascontiguousarray(np.asarray(P, dtype=np.float32))
    d_hw = np.ascontiguousarray(np.asarray(d_hw, dtype=np.int32))
    edge_src = np.ascontiguousarray(np.asarray(edge_src, dtype=np.int32))
    edge_dst = np.ascontiguousarray(np.asarray(edge_dst, dtype=np.int32))
    edge_w = np.ascontiguousarray(np.asarray(edge_w, dtype=np.float32))
    iota = np.broadcast_to(np.arange(512, dtype=np.float16), (128, 512)).copy()
    ones = np.ones((128, 128), dtype=np.float32)
    in_maps = []
    for c in range(NCORES):
        sl = slice(c * BLOC, (c + 1) * BLOC)
        in_maps.append(
            {
                "P": P[sl],
                "d_hw": d_hw,
                "edge_src": edge_src[sl],
                "edge_dst": edge_dst[sl],
                "edge_w": edge_w[sl],
                "iota": iota,
                "ones": ones,
            }
        )
    return in_maps


def kernel(P, d_hw, edge_src, edge_dst, edge_w):
    from concourse.bass_utils import run_bass_kernel_spmd

    if "nc" not in _CACHE:
        _CACHE["nc"] = build()
    nc = _CACHE["nc"]
    in_maps = shard_inputs(P, d_hw, edge_src, edge_dst, edge_w)
    res = run_bass_kernel_spmd(nc, in_maps, core_ids=list(range(NCORES)))
    partial = sum(float(res.results[c]["out"][0, 0]) for c in range(NCORES))
    return np.float32(-partial / B)
